# revision 48
# baseline (speedup 1.0000x reference)
"""DASTNCell Trainium2 kernel — 8-core data-parallel over batch.

Math (per batch b):
  STE = se + c_b                        (host: embedding lookup, [16,N] T-layout)
  E_T[m,n] = exp(STE_m . STE_n + R[n,m] + SC[n,m])        (scores transposed)
  P_T = insaug^T-contract: P_T[c,n] = sum_m insaug[m,c] E_T[m,n]; row 96 = Z
  gate/upd einsums via y-tiles: y_d = ste_d * [state_T ; Astate_hat]
  z,r = sigmoid(gate) = 0.5*tanh(0.5*gate)+0.5 ; hc = tanh(upd)
  out = r*state + (1-r)*hc

Implementation notes:
  - The repeat loop is a hardware For_i, so program size (and per-call NEFF
    load/dispatch cost) is repeat-invariant; the repeat-slope then isolates
    actual per-pass device execution.
  - ste replicated across partitions with a single broadcast DMA from DRAM
    ([1,16,N] -> [128,16,N]) instead of 256 selector matmuls.
  - E / insaug / y path in bf16 (DVE 2x, half DMA), accumulation fp32 PSUM.
"""
import sys

sys.path.insert(0, "/opt/trn_rl_repo")
import numpy as np

_P, _F, _FR = 288, 7, 12
HID = 64
B, N, ET = 32, 1024, 16
NCORES = 8
BPC = B // NCORES  # batches per core
NT = N // 128      # m-tiles
NJ = N // 512      # n-chunks
CH = 512

_cache = {}


def _build(repeat=1, y_gpsimd=False, unroll=1):
    import concourse.bacc as bacc
    import concourse.tile as tile
    from concourse import mybir

    F32 = mybir.dt.float32
    F32R = mybir.dt.float32r
    BF16 = mybir.dt.bfloat16
    AF = mybir.ActivationFunctionType
    OP = mybir.AluOpType

    nc = bacc.Bacc(None, target_bir_lowering=False, debug=False, num_devices=NCORES)

    # ---- DRAM I/O ----
    d_ersc = nc.dram_tensor("ersc", [BPC, N, N], BF16, kind="ExternalInput")
    d_ste = nc.dram_tensor("ste", [BPC, ET, N], F32, kind="ExternalInput")
    d_stebf = nc.dram_tensor("stebf", [BPC, 1, ET, N], BF16, kind="ExternalInput")
    d_insaug = nc.dram_tensor("insaug", [BPC, 128, NT, 97], BF16, kind="ExternalInput")
    d_statet = nc.dram_tensor("statet", [BPC, HID, N], F32, kind="ExternalInput")
    d_xbc = nc.dram_tensor("xbc", [BPC, ET, N], BF16, kind="ExternalInput")
    d_wg = nc.dram_tensor("wg", [128, ET, 128], BF16, kind="ExternalInput")
    d_wu = nc.dram_tensor("wu", [128, ET, HID], BF16, kind="ExternalInput")
    d_wxg = nc.dram_tensor("wxg", [48, 128], BF16, kind="ExternalInput")
    d_wxu = nc.dram_tensor("wxu", [48, HID], BF16, kind="ExternalInput")
    d_bg = nc.dram_tensor("bg", [ET, 128], BF16, kind="ExternalInput")
    d_bu = nc.dram_tensor("bu", [ET, HID], BF16, kind="ExternalInput")
    d_ones64 = nc.dram_tensor("ones64", [1, HID], F32, kind="ExternalInput")
    d_out = nc.dram_tensor("outt", [BPC, HID, N], F32, kind="ExternalOutput")

    with tile.TileContext(nc) as tc:
        with (
            tc.tile_pool(name="consts", bufs=1) as consts,
            tc.tile_pool(name="perb", bufs=2) as perb,
            tc.tile_pool(name="perb1", bufs=1) as perb1,
            tc.tile_pool(name="perb2", bufs=2) as perb2,
            tc.tile_pool(name="epool", bufs=2) as epool,
            tc.tile_pool(name="rscs", bufs=2) as rscs,
            tc.tile_pool(name="reppool", bufs=1) as reppool,
            tc.tile_pool(name="ypool", bufs=3) as ypool,
            tc.tile_pool(name="tpool", bufs=1) as tpool,
            tc.tile_pool(name="ps_sc", bufs=2, space="PSUM") as ps_sc,
            tc.tile_pool(name="ps_np", bufs=2, space="PSUM") as ps_np,
            tc.tile_pool(name="ps_rep", bufs=2, space="PSUM") as ps_rep,
            tc.tile_pool(name="ps_out", bufs=2, space="PSUM") as ps_out,
        ):
            # ---- constants ----
            wg_sb = consts.tile([128, ET, 128], BF16)
            wu_sb = consts.tile([128, ET, HID], BF16)
            wxg_sb = consts.tile([48, 128], BF16)
            wxu_sb = consts.tile([48, HID], BF16)
            bg_sb = consts.tile([ET, 128], BF16)
            bu_sb = consts.tile([ET, HID], BF16)
            o64_sb = consts.tile([1, HID], F32R)
            for sb, dr in ((wg_sb, d_wg), (wu_sb, d_wu), (wxg_sb, d_wxg),
                           (wxu_sb, d_wxu), (bg_sb, d_bg), (bu_sb, d_bu)):
                nc.sync.dma_start(out=sb[:], in_=dr.ap())
            nc.sync.dma_start(out=o64_sb[:], in_=d_ones64.ap().bitcast(F32R))

            MM = nc.tensor.matmul

            def emit_batch(b):
                ste_sb = perb.tile([ET, N], F32R, tag="ste")
                nc.sync.dma_start(out=ste_sb[:], in_=d_ste.ap()[b].bitcast(F32R))
                ia_sb = perb.tile([128, NT, 97], BF16, tag="insaug")
                nc.sync.dma_start(out=ia_sb[:], in_=d_insaug.ap()[b])
                st_sb = perb.tile([HID, N], F32, tag="statet")
                nc.sync.dma_start(out=st_sb[:], in_=d_statet.ap()[b])
                xbc_sb = perb2.tile([ET, N], BF16, tag="xbc")
                nc.sync.dma_start(out=xbc_sb[:], in_=d_xbc.ap()[b])
                E = epool.tile([128, NT, N], BF16, tag="E")

                # ---------- phase 1: scores + exp ----------
                # E = max(exp(G), 1) * exp(RSC)   [= exp(relu(G) + RSC)]
                ersc_sb = rscs.tile([128, NT, N], BF16, tag="rsc")
                nc.sync.dma_start(
                    out=ersc_sb[:],
                    in_=d_ersc.ap()[b].rearrange("(t p) n -> p t n", p=128))
                rep = reppool.tile([128, ET, N], BF16, tag="rep")
                nc.sync.dma_start(
                    out=rep[:], in_=d_stebf.ap()[b].broadcast_to([128, ET, N]))
                for j in range(NJ):
                    for t in range(NT):
                        ps = ps_sc.tile([128, CH], F32, tag="sc")
                        MM(ps[:], ste_sb[:, 128 * t:128 * (t + 1)],
                           ste_sb[:, CH * j:CH * (j + 1)], start=True, stop=True)
                        nc.scalar.activation(
                            out=E[:, t, CH * j:CH * (j + 1)],
                            in_=ps[:], func=AF.Exp)
                for j in range(NJ):
                    cs = slice(CH * j, CH * (j + 1))
                    nc.vector.tensor_scalar(out=E[:, :, cs], in0=E[:, :, cs],
                                            scalar1=1.0, scalar2=None, op0=OP.max)
                    nc.vector.tensor_mul(E[:, :, cs], E[:, :, cs],
                                         ersc_sb[:, :, cs])

                # ---------- phase 2: numerators + rz + u ----------
                rz = perb2.tile([HID, N], F32, tag="rz")
                zrow = perb2.tile([1, N], F32R, tag="zrow")
                axn = perb2.tile([ET, N], BF16, tag="axn")
                u_sb = perb2.tile([48, N], BF16, tag="u")
                xgS = perb.tile([128, N], BF16, tag="xgS")
                stebf_sb = perb2.tile([ET, N], BF16, tag="stebf")
                nc.vector.tensor_copy(out=stebf_sb[:], in_=ste_sb[:].bitcast(F32))
                nc.vector.tensor_copy(out=xgS[0:HID, :], in_=st_sb[:])
                for j in range(NJ):
                    cs = slice(CH * j, CH * (j + 1))
                    pp = ps_np.tile([97, CH], F32, tag="np")
                    for t in range(NT):
                        MM(pp[:], ia_sb[:, t, :], E[:, t, cs],
                           start=(t == 0), stop=(t == NT - 1))
                    # Z row -> SBUF, replicate to 64 partitions, reciprocal
                    nc.scalar.copy(out=zrow[:, cs], in_=pp[96:97, :])
                    zr_ps = ps_rep.tile([HID, CH], F32, tag="rep")
                    MM(zr_ps[:], o64_sb[:], zrow[:, cs], start=True, stop=True)
                    nc.vector.reciprocal_approx_fast(out=rz[:, cs], in_=zr_ps[:])
                    # normalize A@state -> xgS rows 64:128 (bf16)
                    nc.vector.tensor_mul(xgS[HID:128, cs], pp[0:HID, :], rz[:, cs])
                    # normalized A@x replicated rows (pp rows 64:80)
                    nc.vector.tensor_mul(axn[:, cs], pp[HID:HID + ET, :],
                                         rz[0:ET, cs])
                # u rows: 0:16 = ste*x, 32:48 = ste*axhat (16:32 zeroed; the
                # matching wxg/wxu rows are zero but NaN*0 would still poison)
                nc.vector.memset(u_sb[0:32, :], 0.0)
                nc.gpsimd.tensor_mul(u_sb[0:ET, :], stebf_sb[:], xbc_sb[:])
                nc.gpsimd.tensor_mul(u_sb[32:48, :], stebf_sb[:], axn[:])

                # ---------- phase 3: gate ----------
                zrt = perb1.tile([128, N], BF16, tag="zrt")
                z_sb = perb1.tile([HID, N], BF16, tag="z")
                r_sb = perb1.tile([HID, N], F32, tag="r")
                g0 = ps_out.tile([128, CH], F32, tag="out")
                g1 = ps_out.tile([128, CH], F32, tag="out")
                MM(g0[:], bg_sb[:], stebf_sb[:, 0:CH], start=True, stop=False)
                MM(g1[:], bg_sb[:], stebf_sb[:, CH:N], start=True, stop=False)
                MM(g0[:], wxg_sb[:], u_sb[:, 0:CH], start=False, stop=False)
                MM(g1[:], wxg_sb[:], u_sb[:, CH:N], start=False, stop=False)
                for d in range(ET):
                    y_sb = ypool.tile([128, N], BF16, tag="y")
                    yeng = nc.gpsimd if (y_gpsimd and d % 4 == 3) else nc.vector
                    yeng.tensor_mul(y_sb[:], xgS[:], rep[:, d, :])
                    MM(g0[:], wg_sb[:, d, :], y_sb[:, 0:CH],
                       start=False, stop=(d == ET - 1))
                    MM(g1[:], wg_sb[:, d, :], y_sb[:, CH:N],
                       start=False, stop=(d == ET - 1))
                # sigmoid via tanh: z,r = 0.5*tanh(0.5*g)+0.5
                nc.scalar.activation(out=zrt[:, 0:CH], in_=g0[:],
                                     func=AF.Tanh, scale=0.5)
                nc.scalar.activation(out=zrt[:, CH:N], in_=g1[:],
                                     func=AF.Tanh, scale=0.5)
                for j in range(NJ):
                    cs = slice(CH * j, CH * (j + 1))
                    nc.vector.tensor_scalar(out=z_sb[:, cs], in0=zrt[0:HID, cs],
                                            scalar1=0.5, scalar2=0.5,
                                            op0=OP.mult, op1=OP.add)
                    nc.vector.tensor_scalar(out=r_sb[:, cs], in0=zrt[HID:128, cs],
                                            scalar1=0.5, scalar2=0.5,
                                            op0=OP.mult, op1=OP.add)

                # ---------- phase 4: z*state, transpose, numer2 ----------
                # z*state in bf16 (xgS rows 0:64 hold bf16 state), then one
                # bf16 DMA-transpose produces the [128, NT, HID] lhsT layout.
                xgU = perb1.tile([128, N], BF16, tag="xgU")
                for j in range(NJ):
                    cs = slice(CH * j, CH * (j + 1))
                    nc.gpsimd.tensor_mul(xgU[0:HID, cs], z_sb[:, cs],
                                         xgS[0:HID, cs])
                zsn = perb1.tile([128, NT, HID], BF16, tag="zsn")
                nc.scalar.dma_start(out=zsn[:], in_=xgU[0:HID, :], transpose=True)
                for j in range(NJ):
                    cs = slice(CH * j, CH * (j + 1))
                    p2 = ps_np.tile([HID, CH], F32, tag="np")
                    for t in range(NT):
                        MM(p2[:], zsn[:, t, :], E[:, t, cs],
                           start=(t == 0), stop=(t == NT - 1))
                    nc.vector.tensor_mul(xgU[HID:128, cs], p2[:], rz[:, cs])

                # ---------- phase 5: upd + combine ----------
                hc_sb = perb1.tile([HID, N], F32, tag="hc")
                outT = perb1.tile([HID, N], F32, tag="outT")
                u0 = ps_out.tile([HID, CH], F32, tag="out")
                u1 = ps_out.tile([HID, CH], F32, tag="out")
                MM(u0[:], bu_sb[:], stebf_sb[:, 0:CH], start=True, stop=False)
                MM(u1[:], bu_sb[:], stebf_sb[:, CH:N], start=True, stop=False)
                MM(u0[:], wxu_sb[:], u_sb[:, 0:CH], start=False, stop=False)
                MM(u1[:], wxu_sb[:], u_sb[:, CH:N], start=False, stop=False)
                for d in range(ET):
                    y_sb = ypool.tile([128, N], BF16, tag="y")
                    yeng = nc.gpsimd if (y_gpsimd and d % 4 == 3) else nc.vector
                    yeng.tensor_mul(y_sb[:], xgU[:], rep[:, d, :])
                    MM(u0[:], wu_sb[:, d, :], y_sb[:, 0:CH],
                       start=False, stop=(d == ET - 1))
                    MM(u1[:], wu_sb[:, d, :], y_sb[:, CH:N],
                       start=False, stop=(d == ET - 1))
                nc.scalar.activation(out=hc_sb[:, 0:CH], in_=u0[:], func=AF.Tanh)
                nc.scalar.activation(out=hc_sb[:, CH:N], in_=u1[:], func=AF.Tanh)
                # out = hc + r*(state-hc)
                t1 = tpool.tile([HID, N], F32, tag="t1")
                nc.gpsimd.tensor_sub(t1[:], st_sb[:], hc_sb[:])
                nc.gpsimd.tensor_mul(t1[:], t1[:], r_sb[:])
                nc.gpsimd.tensor_add(outT[:], t1[:], hc_sb[:])
                # Output DMA rides the ACT HWDGE queue: on the SP queue its
                # semaphore wait would head-of-line-block the next batch's
                # input DMAs.
                nc.scalar.dma_start(out=d_out.ap()[b], in_=outT[:])

            assert repeat % unroll == 0
            with tc.For_i(0, repeat // unroll):
                for _u in range(unroll):
                    for b in range(BPC):
                        emit_batch(b)

    nc.compile()
    return nc


def _host_prep(inputs):
    import ml_dtypes
    bf16 = ml_dtypes.bfloat16
    f32 = np.float32
    x = np.ascontiguousarray(inputs["x"], f32)
    R = np.ascontiguousarray(inputs["R"], f32)
    state = np.ascontiguousarray(inputs["state"], f32)
    SC = np.ascontiguousarray(inputs["SC"], f32)
    SE = np.ascontiguousarray(inputs["SE"], f32)
    W_se = np.ascontiguousarray(inputs["W_se"], f32)
    b_se = np.ascontiguousarray(inputs["b_se"], f32)
    T_tod = np.ascontiguousarray(inputs["T_tod"], f32)
    T_dow = np.ascontiguousarray(inputs["T_dow"], f32)
    W_gate = np.ascontiguousarray(inputs["W_gate"], f32)
    b_gate = np.ascontiguousarray(inputs["b_gate"], f32)
    W_upd = np.ascontiguousarray(inputs["W_upd"], f32)
    b_upd = np.ascontiguousarray(inputs["b_upd"], f32)
    ti = np.asarray(inputs["time_index"]).astype(np.int64)

    se = SE @ W_se + b_se                            # [N, ET]
    t = ti * _FR
    c = T_tod[t % _P] + T_dow[(t // _P) % _F]        # [B, ET]
    STE_T = np.ascontiguousarray((se[None] + c[:, None]).transpose(0, 2, 1))
    ERSC_T = np.exp(np.ascontiguousarray((R + SC[None]).transpose(0, 2, 1)))
    xrep = np.broadcast_to(x, (B, N, ET))
    insaug = np.concatenate(
        [state, xrep, np.zeros((B, N, ET), f32), np.ones((B, N, 1), f32)], axis=2)
    insaug = np.ascontiguousarray(
        insaug.reshape(B, NT, 128, 97).transpose(0, 2, 1, 3))
    xbc = np.ascontiguousarray(
        np.broadcast_to(x[:, None, :, 0], (B, ET, N))).astype(f32)
    state_T = np.ascontiguousarray(state.transpose(0, 2, 1))

    wg = np.ascontiguousarray(
        np.concatenate([W_gate[:, 0, 1:65, :], W_gate[:, 1, 1:65, :]], axis=1)
        .transpose(1, 0, 2))                          # [128, ET, 128]
    wu = np.ascontiguousarray(
        np.concatenate([W_upd[:, 0, 1:65, :], W_upd[:, 1, 1:65, :]], axis=1)
        .transpose(1, 0, 2))                          # [128, ET, 64]
    zpad_g = np.zeros((ET, 2 * HID), f32)
    zpad_u = np.zeros((ET, HID), f32)
    wxg = np.ascontiguousarray(
        np.concatenate([W_gate[:, 0, 0, :], zpad_g, W_gate[:, 1, 0, :]], axis=0))
    wxu = np.ascontiguousarray(
        np.concatenate([W_upd[:, 0, 0, :], zpad_u, W_upd[:, 1, 0, :]], axis=0))

    shared = {
        "wg": wg.astype(bf16), "wu": wu.astype(bf16),
        "wxg": wxg.astype(bf16), "wxu": wxu.astype(bf16),
        "bg": b_gate.astype(bf16), "bu": b_upd.astype(bf16),
        "ones64": np.ones((1, HID), f32),
    }
    in_maps = []
    for core in range(NCORES):
        bs = slice(BPC * core, BPC * (core + 1))
        m = dict(shared)
        m["ersc"] = np.ascontiguousarray(ERSC_T[bs]).astype(bf16)
        ste_c = np.ascontiguousarray(STE_T[bs])
        m["ste"] = ste_c
        m["stebf"] = ste_c[:, None, :, :].astype(bf16)
        m["insaug"] = np.ascontiguousarray(insaug[bs]).astype(bf16)
        m["statet"] = np.ascontiguousarray(state_T[bs])
        m["xbc"] = np.ascontiguousarray(xbc[bs]).astype(bf16)
        in_maps.append(m)
    return in_maps


def kernel(**inputs):
    from concourse.bass_utils import run_bass_kernel_spmd

    if "nc" not in _cache:
        _cache["nc"] = _build(repeat=1)
    nc = _cache["nc"]
    in_maps = _host_prep(inputs)
    res = run_bass_kernel_spmd(nc, in_maps, core_ids=list(range(NCORES)))
    outs = [r["outt"] for r in res.results]          # each [BPC, 64, N]
    out = np.concatenate(outs, axis=0)               # [B, 64, N]
    return np.ascontiguousarray(out.transpose(0, 2, 1)).astype(np.float32)
